# revision 8
# baseline (speedup 1.0000x reference)
"""Trainium2 Bass kernel for per-pixel bucketed 3x3 conv (RAISR-style).

Problem: out[b,o,h,w] = sum_p patches[b,p,h,w] * W[buckets[b,h,w], o, p] + bias
  B=4, Cin=8, Cout=8, K=3, H=W=256, NUM_TYPES=216 filter buckets.

Strategy (8 NeuronCores, data-parallel over H with k//2 halo):
  - Each core owns 32 rows of H for all 4 batch images: 128 (b,h) pairs ->
    the 128 SBUF partitions; w (256) along the free axis.
  - Host prepares (data-independent layout transforms only):
      * im2col patches, bf16, [128, 256, 80] per core (72 features + ones
        row for bias + pad; k padded to 74-even for DVE 2x mode alignment)
      * the 216-row filter table re-laid to [216, 640] bf16 rows
        (per out-channel 74-element blocks: 72 weights + bias + pad)
      * bucket ids as int16 in the dma_gather "wrapped" index layout
  - Device per core: 16 pipelined iterations over w-blocks:
      dma_gather of 2048 per-pixel weight rows (HBM table -> SBUF, pixel ->
      partition), DVE bf16 multiply vs broadcast patches, DVE segmented
      reduce -> f32 output [128, 8, 256]; single contiguous output DMA.
"""

import numpy as np

B, Cin, Cout, K, H, W = 4, 8, 8, 3, 256, 256
NUM_TYPES = 216
NCORES = 8
RH = H // NCORES          # 32 rows of H per core
P = 128                   # partitions = B * RH
KPAD = 74                 # per-o feature block (72 weights + bias + 1 pad)
ROWLEN = 640              # table row length in bf16 elems (8*74=592 -> pad 640)
PATLEN = 80               # patch row stride per pixel
NPX = B * RH * W          # pixels per core = 32768
GCALLS = 64               # gather calls per core (<=512 idxs per call: 64-desc
                          # per-engine packet limit on the SWDGE ucode path)
IDX_PER_CALL = NPX // GCALLS  # 512
CBLK = IDX_PER_CALL // P      # 4 w-columns per gather call
GROUPS = 16               # DVE op groups (4 gather calls each)
BLKS = W // GROUPS        # 16 w-columns per DVE group

_COMPILED = {}


def _build_nc():
    from concourse import bacc, mybir
    from concourse.tile import TileContext

    nc = bacc.Bacc(None, target_bir_lowering=False, debug=False)
    bf16 = mybir.dt.bfloat16
    pat_ext = nc.declare_dram_parameter("pat", [P, W * PATLEN], bf16, isOutput=False)
    wtab_ext = nc.declare_dram_parameter("wtab", [NUM_TYPES, ROWLEN], bf16, isOutput=False)
    bidx_ext = nc.declare_dram_parameter(
        "bidx", [P, GCALLS * (IDX_PER_CALL // 16)], mybir.dt.int16, isOutput=False
    )
    out_ext = nc.declare_dram_parameter("out", [P, Cout * W], mybir.dt.float32, isOutput=True)

    from concourse import library_config

    with TileContext(nc) as tc:
        with (
            tc.tile_pool(name="main", bufs=1) as mpool,
            tc.tile_pool(name="wg", bufs=3) as wpool,
            tc.tile_pool(name="prod", bufs=2) as ppool,
        ):
            nc.gpsimd.load_library(library_config.mlp)
            pat_sb = mpool.tile([P, W * PATLEN], bf16)
            nc.sync.dma_start(out=pat_sb[:], in_=pat_ext[:, :])
            icols = IDX_PER_CALL // 16  # idx cols per gather call
            bidx_sb = mpool.tile([P, GCALLS * icols], mybir.dt.int16)
            nc.sync.dma_start(out=bidx_sb[:], in_=bidx_ext[:, :])
            out_sb = mpool.tile([P, Cout * W], mybir.dt.float32)

            pat3 = pat_sb[:].rearrange("p (w k) -> p w k", k=PATLEN)
            out3 = out_sb[:].rearrange("p (o w) -> p o w", w=W)

            for c in range(GROUPS):
                wg = wpool.tile([P, BLKS * ROWLEN], bf16, tag="wg")
                wg3 = wg[:].rearrange("p (t f) -> p t f", f=ROWLEN)
                for s in range(BLKS // CBLK):  # 4 gather calls fill one group
                    gc = c * (BLKS // CBLK) + s
                    nc.gpsimd.dma_gather(
                        out_ap=wg3[:, s * CBLK : (s + 1) * CBLK, :],
                        in_ap=wtab_ext[:, :],
                        idxs_ap=bidx_sb[:, gc * icols : (gc + 1) * icols],
                        num_idxs=IDX_PER_CALL,
                        num_idxs_reg=IDX_PER_CALL,
                        elem_size=ROWLEN,
                    )
                prod = ppool.tile([P, BLKS * Cout * KPAD], bf16, tag="prod")
                prod4 = prod[:].rearrange("p (t o k) -> p t o k", o=Cout, k=KPAD)
                pat_b = (
                    pat3[:, c * BLKS : (c + 1) * BLKS, :KPAD]
                    .unsqueeze(2)
                    .broadcast_to([P, BLKS, Cout, KPAD])
                )
                wg4 = (
                    wg[:]
                    .rearrange("p (t f) -> p t f", f=ROWLEN)[:, :, : Cout * KPAD]
                    .rearrange("p t (o k) -> p t o k", k=KPAD)
                )
                nc.vector.tensor_tensor(
                    out=prod4, in0=pat_b, in1=wg4, op=mybir.AluOpType.mult
                )
                nc.vector.tensor_reduce(
                    out=out3[:, :, c * BLKS : (c + 1) * BLKS].transpose([0, 2, 1]),
                    in_=prod4,
                    axis=mybir.AxisListType.X,
                    op=mybir.AluOpType.add,
                )

            nc.sync.dma_start(out=out_ext[:, :], in_=out_sb[:])
    nc.compile()
    return nc


def _prep_inputs(x, filter_emb, buckets):
    """Host-side data-independent layout prep. Returns in_maps for 8 cores."""
    import ml_dtypes

    bf16 = ml_dtypes.bfloat16
    x = np.asarray(x, dtype=np.float32)
    filter_emb = np.asarray(filter_emb, dtype=np.float32)
    buckets = np.asarray(buckets).astype(np.int64)

    # --- weight table: [216, 640] bf16, row = per-o 74-blocks ---
    nw = Cout * Cin * K * K
    wtab = np.zeros((NUM_TYPES, ROWLEN), dtype=np.float32)
    wmat = filter_emb[:, :nw].reshape(NUM_TYPES, Cout, Cin * K * K)
    bias = filter_emb[:, nw:]  # [216, 8]
    for o in range(Cout):
        wtab[:, o * KPAD : o * KPAD + 72] = wmat[:, o, :]
        wtab[:, o * KPAD + 72] = bias[:, o]
    wtab = wtab.astype(bf16)

    # --- im2col patches, feature order (c, kh, kw) ---
    xp = np.pad(x, ((0, 0), (0, 0), (1, 1), (1, 1)))
    sw = np.lib.stride_tricks.sliding_window_view(xp, (K, K), axis=(2, 3))
    # sw: [B, Cin, H, W, K, K] -> [B, H, W, Cin*K*K]
    patches = sw.transpose(0, 2, 3, 1, 4, 5).reshape(B, H, W, Cin * K * K)

    in_maps = []
    for ci in range(NCORES):
        h0 = ci * RH
        # pat [128=(b,hl), W, 80]
        pat = np.zeros((P, W, PATLEN), dtype=np.float32)
        pslab = patches[:, h0 : h0 + RH]  # [B, RH, W, 72]
        pat[:, :, :72] = pslab.reshape(P, W, 72)
        pat[:, :, 72] = 1.0
        pat = pat.astype(bf16).reshape(P, W * PATLEN)

        # bucket ids in dma_gather wrapped layout
        bcore = buckets[:, h0 : h0 + RH].reshape(P, W).astype(np.int16)
        # gather call gc covers w in [gc*CBLK, (gc+1)*CBLK); position i in the
        # call -> pixel (part=i%128, w = gc*CBLK + i//128); idx position i
        # lives at [partition i%16, col i//16], replicated across the 8
        # 16-partition groups
        icols = IDX_PER_CALL // 16
        bidx = np.empty((P, GCALLS, icols), dtype=np.int16)
        pmat = np.arange(P)[:, None] % 16  # [P,1]
        imat = np.arange(icols)[None, :] * 16 + pmat  # [P, icols] position i
        for gc in range(GCALLS):
            part = imat % P
            wcol = gc * CBLK + imat // P
            bidx[:, gc, :] = bcore[part, wcol]
        bidx = bidx.reshape(P, GCALLS * icols)

        in_maps.append({"pat": pat, "wtab": wtab, "bidx": bidx})
    return in_maps


def kernel(x, filter_emb, buckets):
    from concourse.bass_utils import run_bass_kernel_spmd

    if "nc" not in _COMPILED:
        _COMPILED["nc"] = _build_nc()
    nc = _COMPILED["nc"]

    in_maps = _prep_inputs(x, filter_emb, buckets)
    res = run_bass_kernel_spmd(nc, in_maps, core_ids=list(range(NCORES)))

    out = np.empty((B, Cout, H, W), dtype=np.float32)
    for ci in range(NCORES):
        o = np.asarray(res.results[ci]["out"], dtype=np.float32).reshape(P, Cout, W)
        # partition p = (b = p//RH, hl = p%RH)
        out[:, :, ci * RH : (ci + 1) * RH, :] = o.reshape(B, RH, Cout, W).transpose(
            0, 2, 1, 3
        )
    return out


# revision 13
# speedup vs baseline: 1.1956x; 1.1956x over previous
"""Trainium2 Bass kernel for per-pixel bucketed 3x3 conv (RAISR-style).

Problem: out[b,o,h,w] = sum_p patches[b,p,h,w] * W[buckets[b,h,w], o, p] + bias
  B=4, Cin=8, Cout=8, K=3, H=W=256, NUM_TYPES=216 filter buckets.

Strategy (8 NeuronCores, data-parallel over H with k//2 halo):
  - Each core owns 32 rows of H for all 4 batch images: 128 (b,h) pairs ->
    the 128 SBUF partitions; w (256) along the free axis.
  - Host prepares (data-independent layout transforms only):
      * im2col patches, bf16, [128, 256, 80] per core (72 features + ones
        row for bias + pad; k padded to 74-even for DVE 2x mode alignment)
      * the 216-row filter table re-laid to [216, 640] bf16 rows
        (per out-channel 74-element blocks: 72 weights + bias + pad)
      * bucket ids as int16 in the dma_gather "wrapped" index layout
  - Device per core: 16 pipelined iterations over w-blocks:
      dma_gather of 2048 per-pixel weight rows (HBM table -> SBUF, pixel ->
      partition), DVE bf16 multiply vs broadcast patches, DVE segmented
      reduce -> f32 output [128, 8, 256]; single contiguous output DMA.
"""

import numpy as np

B, Cin, Cout, K, H, W = 4, 8, 8, 3, 256, 256
NUM_TYPES = 216
NCORES = 8
RH = H // NCORES          # 32 rows of H per core
P = 128                   # partitions = B * RH
KPAD = 74                 # per-o feature block (72 weights + bias + 1 pad)
ROWLEN = 640              # table row length in bf16 elems (8*74=592 -> pad 640)
PATLEN = 80               # patch row stride per pixel
NPX = B * RH * W          # pixels per core = 32768
GROUPS = 16               # gather calls / DVE op groups per core
IDX_PER_CALL = NPX // GROUPS  # 2048 (needs single_packet=False: >64
                              # descriptors per DMA engine per call)
BLKS = W // GROUPS        # 16 w-columns per group

_COMPILED = {}


def _build_nc():
    from concourse import bacc, mybir
    from concourse.tile import TileContext

    nc = bacc.Bacc(None, target_bir_lowering=False, debug=False)
    bf16 = mybir.dt.bfloat16
    pat_ext = nc.declare_dram_parameter("pat", [P, W * PATLEN], bf16, isOutput=False)
    wtab_ext = nc.declare_dram_parameter("wtab", [NUM_TYPES, ROWLEN], bf16, isOutput=False)
    bidx_ext = nc.declare_dram_parameter(
        "bidx", [P, GROUPS * (IDX_PER_CALL // 16)], mybir.dt.int16, isOutput=False
    )
    out_ext = nc.declare_dram_parameter("out", [P, Cout * W], mybir.dt.float32, isOutput=True)

    from concourse import library_config

    with TileContext(nc) as tc:
        with (
            tc.tile_pool(name="main", bufs=1) as mpool,
            tc.tile_pool(name="wg", bufs=3) as wpool,
            tc.tile_pool(name="prod", bufs=2) as ppool,
        ):
            nc.gpsimd.load_library(library_config.mlp)
            icols = IDX_PER_CALL // 16  # idx cols per gather call
            bidx_sb = mpool.tile([P, GROUPS * icols], mybir.dt.int16)
            nc.sync.dma_start(out=bidx_sb[:], in_=bidx_ext[:, :])
            pat_sb = mpool.tile([P, W * PATLEN], bf16)
            qpat = W * PATLEN // 4
            for q in range(4):
                nc.sync.dma_start(
                    out=pat_sb[:, q * qpat : (q + 1) * qpat],
                    in_=pat_ext[:, q * qpat : (q + 1) * qpat],
                )
            out_sb = mpool.tile([P, Cout * W], mybir.dt.float32)

            pat3 = pat_sb[:].rearrange("p (w k) -> p w k", k=PATLEN)
            out3 = out_sb[:].rearrange("p (o w) -> p o w", w=W)

            for c in range(GROUPS):
                wg = wpool.tile([P, BLKS * ROWLEN], bf16, tag="wg")
                nc.gpsimd.dma_gather(
                    out_ap=wg[:].rearrange("p (t f) -> p t f", f=ROWLEN),
                    in_ap=wtab_ext[:, :],
                    idxs_ap=bidx_sb[:, c * icols : (c + 1) * icols],
                    num_idxs=IDX_PER_CALL,
                    num_idxs_reg=IDX_PER_CALL,
                    elem_size=ROWLEN,
                    single_packet=False,
                )
                prod = ppool.tile([P, BLKS * Cout * KPAD], bf16, tag="prod")
                prod4 = prod[:].rearrange("p (t o k) -> p t o k", o=Cout, k=KPAD)
                pat_b = (
                    pat3[:, c * BLKS : (c + 1) * BLKS, :KPAD]
                    .unsqueeze(2)
                    .broadcast_to([P, BLKS, Cout, KPAD])
                )
                wg4 = (
                    wg[:]
                    .rearrange("p (t f) -> p t f", f=ROWLEN)[:, :, : Cout * KPAD]
                    .rearrange("p t (o k) -> p t o k", k=KPAD)
                )
                nc.vector.tensor_tensor(
                    out=prod4, in0=pat_b, in1=wg4, op=mybir.AluOpType.mult
                )
                nc.vector.tensor_reduce(
                    out=out3[:, :, c * BLKS : (c + 1) * BLKS].transpose([0, 2, 1]),
                    in_=prod4,
                    axis=mybir.AxisListType.X,
                    op=mybir.AluOpType.add,
                )

                if c % 4 == 3:  # drain finished w-range to HBM
                    q = c // 4
                    oext3 = out_ext[:, :].rearrange("p (o w) -> p o w", w=W)
                    nc.sync.dma_start(
                        out=oext3[:, :, q * 64 : (q + 1) * 64],
                        in_=out3[:, :, q * 64 : (q + 1) * 64],
                    )
    nc.compile()
    return nc


def _prep_inputs(x, filter_emb, buckets):
    """Host-side data-independent layout prep. Returns in_maps for 8 cores."""
    import ml_dtypes

    bf16 = ml_dtypes.bfloat16
    x = np.asarray(x, dtype=np.float32)
    filter_emb = np.asarray(filter_emb, dtype=np.float32)
    buckets = np.asarray(buckets).astype(np.int64)

    # --- weight table: [216, 640] bf16, row = per-o 74-blocks ---
    nw = Cout * Cin * K * K
    wtab = np.zeros((NUM_TYPES, ROWLEN), dtype=np.float32)
    wmat = filter_emb[:, :nw].reshape(NUM_TYPES, Cout, Cin * K * K)
    bias = filter_emb[:, nw:]  # [216, 8]
    for o in range(Cout):
        wtab[:, o * KPAD : o * KPAD + 72] = wmat[:, o, :]
        wtab[:, o * KPAD + 72] = bias[:, o]
    wtab = wtab.astype(bf16)

    # --- im2col patches, feature order (c, kh, kw) ---
    xp = np.pad(x, ((0, 0), (0, 0), (1, 1), (1, 1)))
    sw = np.lib.stride_tricks.sliding_window_view(xp, (K, K), axis=(2, 3))
    # sw: [B, Cin, H, W, K, K] -> [B, H, W, Cin*K*K]
    patches = sw.transpose(0, 2, 3, 1, 4, 5).reshape(B, H, W, Cin * K * K)

    in_maps = []
    for ci in range(NCORES):
        h0 = ci * RH
        # pat [128=(b,hl), W, 80]
        pat = np.zeros((P, W, PATLEN), dtype=np.float32)
        pslab = patches[:, h0 : h0 + RH]  # [B, RH, W, 72]
        pat[:, :, :72] = pslab.reshape(P, W, 72)
        pat[:, :, 72] = 1.0
        pat = pat.astype(bf16).reshape(P, W * PATLEN)

        # bucket ids in dma_gather wrapped layout
        bcore = buckets[:, h0 : h0 + RH].reshape(P, W).astype(np.int16)
        # gather call c covers w in [c*BLKS, (c+1)*BLKS); position i in the
        # call -> pixel (part=i%128, w = c*BLKS + i//128); idx position i
        # lives at [partition i%16, col i//16], replicated across the 8
        # 16-partition groups
        icols = IDX_PER_CALL // 16
        bidx = np.empty((P, GROUPS, icols), dtype=np.int16)
        pmat = np.arange(P)[:, None] % 16  # [P,1]
        imat = np.arange(icols)[None, :] * 16 + pmat  # [P, icols] position i
        for c in range(GROUPS):
            part = imat % P
            wcol = c * BLKS + imat // P
            bidx[:, c, :] = bcore[part, wcol]
        bidx = bidx.reshape(P, GROUPS * icols)

        in_maps.append({"pat": pat, "wtab": wtab, "bidx": bidx})
    return in_maps


def kernel(x, filter_emb, buckets):
    from concourse.bass_utils import run_bass_kernel_spmd

    if "nc" not in _COMPILED:
        _COMPILED["nc"] = _build_nc()
    nc = _COMPILED["nc"]

    in_maps = _prep_inputs(x, filter_emb, buckets)
    res = run_bass_kernel_spmd(nc, in_maps, core_ids=list(range(NCORES)))

    out = np.empty((B, Cout, H, W), dtype=np.float32)
    for ci in range(NCORES):
        o = np.asarray(res.results[ci]["out"], dtype=np.float32).reshape(P, Cout, W)
        # partition p = (b = p//RH, hl = p%RH)
        out[:, :, ci * RH : (ci + 1) * RH, :] = o.reshape(B, RH, Cout, W).transpose(
            0, 2, 1, 3
        )
    return out
